# revision 35
# baseline (speedup 1.0000x reference)
"""Trainium2 Bass kernel for nn_ComplexEncoder (complex-QK transformer encoder layer).

Sharding: 8 cores = (batch b in 0..3) x (seq half h in 0..1). Each core
computes the full output rows for its (b, 512-row) slice. No collectives:
only the K/V projections are duplicated between the two cores of a batch.

v2 restructure (vs baseline): bf16 matmul operands everywhere (PE rate is
unchanged at 512-wide, but DMA/DVE/ACT all halve), softmax denominators
batched into one [16,512] reciprocal off the PE critical path, LayerNorm
via bn_stats + ACT-engine normalize with g1/beta1 folded into W1/b1 on the
host, adds/copies spread across GPSIMD/ACT so the PE queue never waits on
slow DVE work, exp output in bf16, W2 streamed once with 8 PSUM banks.

Math per core (b, half), all matmuls bf16 -> fp32 PSUM:
  qcat^T[128n+j, q] = scale^2-folded q-proj (+ pos folded on host)
  kcat^T[128n+j, k] = (Wk_r | -Wk_i) proj + (pos_k_r | -pos_k_i)
  scores^T[k, q]    = kcat_n^T.T-slices @ qcat_n  (single K=128 contraction)
  sT = exp(scores^T);  PV psum[0:65] via v_aug (col 64 = ones -> row sums)
  headsT = PV[0:64] * bcast(1/PV[64])  (batched recip + K=1 ones matmul)
  attn[q, h] = headsT.T @ Wo;  y1 = attn + (x + bo_eff);  yhat1 = LN1 core
  h1T = PE-transpose(yhat1); ffT = relu(W1'.T @ h1T + b1')   [g1 folded]
  y2 = ffT.T @ W2 + yhat1*g1 + (beta1+b2);  out = LN2(y2)*g2 + beta2
"""

import numpy as np
import ml_dtypes

import concourse.bass as bass
import concourse.bacc as bacc
import concourse.mybir as mybir
import concourse.tile as tile
from concourse.bass_utils import run_bass_kernel_spmd
from concourse.masks import make_identity

F32 = mybir.dt.float32
F32R = mybir.dt.float32r
BF16 = mybir.dt.bfloat16
AF = mybir.ActivationFunctionType
ALU = mybir.AluOpType
AX = mybir.AxisListType

B, S, H, NH, D, FF = 4, 1024, 1024, 16, 64, 4096
SQ = 512  # queries per core
EPS = 1e-5
SCALE = 1.0 / 8.0
HC = H // 128  # 8 chunks of the hidden/contraction dim
FC = FF // 128  # 32 chunks of the ff dim
QC = SQ // 128  # 4 query chunks
KC = S // 128  # 8 key chunks

_CACHE = {}


def build():
    nc = bacc.Bacc(
        "TRN2", target_bir_lowering=False, debug=False,
        enable_asserts=True, num_devices=8,
    )
    # --- DRAM parameters (host-prepped layouts) ---
    dp = nc.declare_dram_parameter
    xT = dp("xT", [128, HC, S], BF16, isOutput=False)       # x[b].T chunked
    xTq = dp("xTq", [128, HC, SQ], BF16, isOutput=False)    # q-half of xT
    wq = dp("wq", [NH, 128, HC * 128], BF16, isOutput=False)
    wk = dp("wk", [NH, 128, HC * 128], BF16, isOutput=False)
    wv = dp("wv", [HC, 128, H], BF16, isOutput=False)
    posq = dp("posq", [NH, 128, SQ], F32, isOutput=False)
    posk = dp("posk", [NH, 128, S], F32, isOutput=False)
    wo = dp("wo", [HC, 128, H], BF16, isOutput=False)
    xqb = dp("xqb", [QC, 128, H], F32, isOutput=False)      # x + bo_eff
    w1 = dp("w1", [FC, 128, HC * 128], BF16, isOutput=False)  # g1 folded
    w2 = dp("w2", [FC, 128, H], BF16, isOutput=False)
    b1c = dp("b1c", [128, FC], F32, isOutput=False)         # b1 + be1@W1
    cns = dp("cns", [128, 4, H], BF16, isOutput=False)      # g1,c1,g2,be2
    bce = dp("bce", [2, 128], F32R, isOutput=False)         # pair selector
    out = dp("out", [QC, 128, H], F32, isOutput=True)

    with tile.TileContext(nc) as tc:
        with (
            tc.tile_pool(name="const", bufs=1) as cp,
            tc.tile_pool(name="psum", bufs=1, space="PSUM") as pp,
            tc.tile_pool(name="persist", bufs=1) as lp,
        ):
            ident = cp.tile([128, 128], BF16)
            make_identity(nc, ident)
            eps_t = cp.tile([128, 1], F32)
            nc.vector.memset(eps_t, EPS)
            bceA = cp.tile([1, 128], F32R)
            nc.sync.dma_start(out=bceA, in_=bce[0:1, :])
            bceB = cp.tile([1, 128], F32R)
            nc.sync.dma_start(out=bceB, in_=bce[1:2, :])
            # b1/cns DMAs deferred below the first attention loads so the
            # head-0 weights win the DMA queue
            b1_sb = cp.tile([128, FC], F32)
            cns_sb = cp.tile([128, 4, H], BF16)
            g1b, c1b = cns_sb[:, 0, :], cns_sb[:, 1, :]
            g2b, be2b = cns_sb[:, 2, :], cns_sb[:, 3, :]

            hT_un = lp.tile([128, HC, SQ], BF16)   # unnormalized heads^T
            headsT = lp.tile([128, HC, SQ], BF16)  # normalized heads^T
            h1n = [lp.tile([128, H], BF16, tag="h1n", bufs=QC, name="h1n")
                   for _ in range(QC)]             # yhat1 (LN1 core out)
            h1gc = [lp.tile([128, H], F32, tag="h1gc", bufs=QC, name="h1gc")
                    for _ in range(QC)]            # yhat1*g1 + (be1+b2)
            h1T = lp.tile([128, HC, SQ], BF16)

            def ps():
                return pp.tile([128, 512], F32, tag="ps", bufs=8, name="pst")

            # ================= attention =================
            with tc.tile_pool(name="attn", bufs=1) as ap:
                xt_sb = ap.tile([128, HC, S], BF16)
                xtq_sb = ap.tile([128, HC, SQ], BF16)
                rr_r = [ap.tile([1, 512], F32R, tag="rr_r", bufs=NH,
                                name="rr_r") for _ in range(NH)]

                def dma_head(n):
                    wkt = ap.tile([128, HC * 128], BF16, tag="wkt", bufs=2,
                                  name="wkt")
                    nc.sync.dma_start(out=wkt, in_=wk[n, :, :])
                    wqt = ap.tile([128, HC * 128], BF16, tag="wqt", bufs=2,
                                  name="wqt")
                    nc.sync.dma_start(out=wqt, in_=wq[n, :, :])
                    pkt = ap.tile([128, S], F32, tag="pkt", bufs=2, name="pkt")
                    nc.sync.dma_start(out=pkt, in_=posk[n, :, :])
                    pqt = ap.tile([128, SQ], F32, tag="pqt", bufs=2,
                                  name="pqt")
                    nc.sync.dma_start(out=pqt, in_=posq[n, :, :])
                    return wkt, wqt, pkt, pqt

                def kq_thunks(wkt, wqt, pk0, pk1, pq):
                    th = []
                    for hc in range(HC):
                        st = wkt[:, hc * 128:(hc + 1) * 128]
                        th.append((pk0, st, xt_sb[:, hc, 0:512],
                                   hc == 0, hc == HC - 1))
                        th.append((pk1, st, xt_sb[:, hc, 512:1024],
                                   hc == 0, hc == HC - 1))
                    for hc in range(HC):
                        th.append((pq, wqt[:, hc * 128:(hc + 1) * 128],
                                   xtq_sb[:, hc, :], hc == 0, hc == HC - 1))
                    return th

                def run_thunk(t):
                    nc.tensor.matmul(t[0][:], t[1], t[2], start=t[3],
                                     stop=t[4])

                def kq_adds(pk0, pk1, pq, pkt, pqt):
                    # order: kcat half 1, qcat, kcat half 2 — the next head's
                    # first score matmul needs only the first two
                    kcat = ap.tile([128, S], BF16, tag="kcat", bufs=2,
                                   name="kcat")
                    nc.vector.tensor_add(kcat[:, 0:512], pk0[:], pkt[:, 0:512])
                    qcat = ap.tile([128, SQ], BF16, tag="qcat", bufs=2,
                                   name="qcat")
                    nc.vector.tensor_add(qcat[:], pq[:], pqt[:])
                    nc.vector.tensor_add(kcat[:, 512:1024], pk1[:],
                                         pkt[:, 512:1024])
                    return kcat, qcat

                def emit_pair(j):
                    # two K=1 outer products: head 2j's 1/denom row ->
                    # partitions 0:64, head 2j+1's -> 64:128 of one bank
                    pbc = ps()
                    nc.tensor.matmul(pbc[:], bceA[:], rr_r[2 * j][:],
                                     start=True, stop=False)
                    nc.tensor.matmul(pbc[:], bceB[:], rr_r[2 * j + 1][:],
                                     start=False, stop=True)
                    bc_sb = ap.tile([128, SQ], BF16, tag="bc", bufs=2,
                                    name="bc")
                    nc.scalar.copy(bc_sb[:], pbc[:])
                    nc.vector.tensor_mul(
                        headsT[:, j, :], hT_un[:, j, :], bc_sb[:])

                # prologue: head-0 weights first in the DMA queue, then x
                hw = dma_head(0)
                for hc in range(HC):
                    nc.sync.dma_start(out=xt_sb[:, hc, :], in_=xT[:, hc, :])
                for hc in range(HC):
                    nc.sync.dma_start(out=xtq_sb[:, hc, :], in_=xTq[:, hc, :])
                pk0, pk1, pq = ps(), ps(), ps()
                for t in kq_thunks(hw[0], hw[1], pk0, pk1, pq):
                    run_thunk(t)
                kcat, qcat = kq_adds(pk0, pk1, pq, hw[2], hw[3])
                # consts ride behind the startup-critical loads
                nc.sync.dma_start(out=b1_sb, in_=b1c[:, :])
                nc.sync.dma_start(out=cns_sb, in_=cns[:, :, :])

                # V projection (overlaps head-0 weight/pos DMAs)
                v_sb = []
                for kc in range(KC):
                    va = ap.tile([128, NH, 65], BF16, tag="vaug", bufs=KC,
                                 name="va")
                    v_sb.append(va)
                with tc.tile_pool(name="vw", bufs=1) as vwp:
                    wv_sb = []
                    for hc in range(HC):
                        wvt = vwp.tile([128, H], BF16, tag="wv", bufs=HC,
                                       name="wvt")
                        nc.sync.dma_start(out=wvt, in_=wv[hc, :, :])
                        wv_sb.append(wvt)
                    for kc in range(KC):
                        p0, p1 = ps(), ps()
                        for hc in range(HC):
                            st = xt_sb[:, hc, kc * 128:(kc + 1) * 128]
                            nc.tensor.matmul(p0[:], st, wv_sb[hc][:, 0:512],
                                             start=(hc == 0),
                                             stop=(hc == HC - 1))
                            nc.tensor.matmul(p1[:], st, wv_sb[hc][:, 512:1024],
                                             start=(hc == 0),
                                             stop=(hc == HC - 1))
                        nc.vector.tensor_copy(
                            v_sb[kc][:, 0:8, 0:64],
                            p0[:].rearrange("p (n d) -> p n d", d=64))
                        nc.scalar.copy(
                            v_sb[kc][:, 8:16, 0:64],
                            p1[:].rearrange("p (n d) -> p n d", d=64))
                        nc.gpsimd.memset(v_sb[kc][:, :, 64:65], 1.0)

                # per-head: scores/exp/PV for head n interleaved (in the PE
                # queue) with head n+1's K/Q projection matmuls
                pairq = []
                for n in range(NH):
                    last = (n == NH - 1)
                    th = []
                    if not last:
                        hw_n = dma_head(n + 1)
                        pk0n, pk1n, pqn = ps(), ps(), ps()
                        th = kq_thunks(hw_n[0], hw_n[1], pk0n, pk1n, pqn)
                    ppv = ps()
                    psc_prev = ps()
                    nc.tensor.matmul(psc_prev[:], kcat[:, 0:128], qcat[:],
                                     start=True, stop=True)
                    ti = 0
                    for kc in range(KC):
                        sT = ap.tile([128, SQ], BF16, tag="sT", bufs=3,
                                     name="sT")
                        nc.scalar.activation(sT[:], psc_prev[:], AF.Exp)
                        if kc < KC - 1:
                            psc_next = ps()
                            nc.tensor.matmul(
                                psc_next[:],
                                kcat[:, (kc + 1) * 128:(kc + 2) * 128],
                                qcat[:], start=True, stop=True)
                        nc.tensor.matmul(ppv[0:65, :], v_sb[kc][:, n, :],
                                         sT[:], start=(kc == 0),
                                         stop=(kc == KC - 1))
                        for _ in range(3):
                            if ti < len(th):
                                run_thunk(th[ti])
                                ti += 1
                        if kc < KC - 1:
                            psc_prev = psc_next
                    while ti < len(th):
                        run_thunk(th[ti])
                        ti += 1

                    off = (n % 2) * 64
                    nc.scalar.copy(hT_un[off:off + 64, n // 2, :],
                                   ppv[0:64, :])
                    if not last:
                        kcat, qcat = kq_adds(pk0n, pk1n, pqn, hw_n[2],
                                             hw_n[3])
                    rsc = ap.tile([1, 512], F32, tag="rsc", bufs=3,
                                  name="rsc")
                    nc.vector.reciprocal(rsc[:], ppv[64:65, :])
                    with nc.allow_low_precision(reason="denom fp32r"):
                        nc.scalar.copy(rr_r[n][:], rsc[:])
                    # normalize pairs trickle in two heads behind the recips
                    if n % 2 == 1:
                        pairq.append(n // 2)
                        if len(pairq) > 1:
                            emit_pair(pairq.pop(0))
                for j in pairq:
                    emit_pair(j)

            # ================= output proj + LN1 =================
            with tc.tile_pool(name="wop", bufs=1) as wop:
                wo_sb = []
                for j in range(HC):
                    wot = wop.tile([128, H], BF16, tag="wot", bufs=HC,
                                   name="wot")
                    nc.sync.dma_start(out=wot, in_=wo[j, :, :])
                    wo_sb.append(wot)

                def wo_block(qc):
                    po0, po1 = ps(), ps()
                    for j in range(HC):
                        st = headsT[:, j, qc * 128:(qc + 1) * 128]
                        nc.tensor.matmul(po0[:], st, wo_sb[j][:, 0:512],
                                         start=(j == 0), stop=(j == HC - 1))
                        nc.tensor.matmul(po1[:], st, wo_sb[j][:, 512:1024],
                                         start=(j == 0), stop=(j == HC - 1))
                    xqt = wop.tile([128, H], F32, tag="xqt", bufs=2,
                                   name="xqt")
                    nc.sync.dma_start(out=xqt, in_=xqb[qc, :, :])
                    y1 = wop.tile([128, H], F32, tag="y1", bufs=2, name="y1")
                    nc.vector.tensor_add(y1[:, 0:512], po0[:], xqt[:, 0:512])
                    nc.vector.tensor_add(y1[:, 512:1024], po1[:],
                                         xqt[:, 512:1024])
                    # LN1 core: yhat = (y1 - mu) * rstd  (g1/be1 folded out)
                    st6 = wop.tile([128, 2, 6], F32, tag="st6", bufs=2,
                                   name="st6")
                    nc.vector.bn_stats(st6[:, 0, :], y1[:, 0:512])
                    nc.vector.bn_stats(st6[:, 1, :], y1[:, 512:1024])
                    mv = wop.tile([128, 2], F32, tag="mv", bufs=2, name="mv")
                    nc.vector.bn_aggr(mv[:], st6[:])
                    std = wop.tile([128, 1], F32, tag="std", bufs=2,
                                   name="std")
                    nc.scalar.activation(std[:], mv[:, 1:2], AF.Sqrt,
                                         bias=eps_t[:])
                    rstd = wop.tile([128, 1], F32, tag="rstd", bufs=2,
                                    name="rstd")
                    nc.vector.reciprocal(rstd[:], std[:])
                    nmr = wop.tile([128, 1], F32, tag="nmr", bufs=2,
                                   name="nmr")
                    nc.vector.scalar_tensor_tensor(
                        nmr[:], mv[:, 0:1], -1.0, rstd[:],
                        op0=ALU.mult, op1=ALU.mult)
                    nc.scalar.activation(h1n[qc][:], y1[:], AF.Identity,
                                         bias=nmr[:], scale=rstd[:])
                    # residual term on idle gpsimd: h1gc = yhat*g1 + (be1+b2)
                    tres = wop.tile([128, H], BF16, tag="tres", bufs=2,
                                    name="tres")
                    nc.gpsimd.tensor_mul(tres[:], h1n[qc][:], g1b)
                    nc.gpsimd.tensor_add(h1gc[qc][:], tres[:], c1b)

                def tr_block(qc):
                    for j in range(HC):
                        pt = ps()
                        ptb = pt[:].bitcast(BF16)[:, 0:128]
                        nc.tensor.transpose(
                            ptb, h1n[qc][:, j * 128:(j + 1) * 128], ident[:])
                        nc.vector.tensor_copy(
                            h1T[:, j, qc * 128:(qc + 1) * 128], ptb)

                # stagger transposes two Wo blocks behind, hiding the LN chain
                wo_block(0)
                wo_block(1)
                wo_block(2)
                tr_block(0)
                wo_block(3)
                tr_block(1)
                tr_block(2)
                tr_block(3)

            # ================= FFN =================
            with tc.tile_pool(name="ffn", bufs=1) as fp:
                ffT = fp.tile([128, FC, SQ], BF16, name="ffT")
                w2sb = [fp.tile([128, H], BF16, tag="w2r", bufs=FC,
                                name="w2r") for _ in range(FC)]
                for f in range(FC):
                    w1t = fp.tile([128, HC * 128], BF16, tag="w1t", bufs=3,
                                  name="w1t")
                    nc.sync.dma_start(out=w1t, in_=w1[f, :, :])
                    # W2 row rides the DMA queue behind this step's W1 row,
                    # landing fully by the time FFN2 needs it
                    nc.sync.dma_start(out=w2sb[f], in_=w2[f, :, :])
                    pf = ps()
                    for hc in range(HC):
                        nc.tensor.matmul(pf[:], w1t[:, hc * 128:(hc + 1) * 128],
                                         h1T[:, hc, :],
                                         start=(hc == 0), stop=(hc == HC - 1))
                    nc.scalar.activation(ffT[:, f, :], pf[:], AF.Relu,
                                         bias=b1_sb[:, f:f + 1])

                # FFN2 per qc: each qc's LN2 tail hides under the next qc's
                # matmul stream
                for qc in range(QC):
                    py0, py1 = ps(), ps()
                    for f in range(FC):
                        st = ffT[:, f, qc * 128:(qc + 1) * 128]
                        nc.tensor.matmul(py0[:], st, w2sb[f][:, 0:512],
                                         start=(f == 0), stop=(f == FC - 1))
                        nc.tensor.matmul(py1[:], st, w2sb[f][:, 512:1024],
                                         start=(f == 0), stop=(f == FC - 1))
                    y2 = fp.tile([128, H], F32, tag="y2", bufs=2, name="y2")
                    nc.vector.tensor_add(y2[:, 0:512], py0[:],
                                         h1gc[qc][:, 0:512])
                    nc.vector.tensor_add(y2[:, 512:1024], py1[:],
                                         h1gc[qc][:, 512:1024])
                    st6 = fp.tile([128, 2, 6], F32, tag="st6b", bufs=2,
                                  name="st6b")
                    nc.vector.bn_stats(st6[:, 0, :], y2[:, 0:512])
                    nc.vector.bn_stats(st6[:, 1, :], y2[:, 512:1024])
                    mv = fp.tile([128, 2], F32, tag="mvb", bufs=2, name="mvb")
                    nc.vector.bn_aggr(mv[:], st6[:])
                    std = fp.tile([128, 1], F32, tag="stdb", bufs=2,
                                  name="stdb")
                    nc.scalar.activation(std[:], mv[:, 1:2], AF.Sqrt,
                                         bias=eps_t[:])
                    rstd = fp.tile([128, 1], F32, tag="rstdb", bufs=2,
                                   name="rstdb")
                    nc.vector.reciprocal(rstd[:], std[:])
                    nmr = fp.tile([128, 1], F32, tag="nmrb", bufs=2,
                                  name="nmrb")
                    nc.vector.scalar_tensor_tensor(
                        nmr[:], mv[:, 0:1], -1.0, rstd[:],
                        op0=ALU.mult, op1=ALU.mult)
                    hy2 = fp.tile([128, H], BF16, tag="hy2", bufs=2,
                                  name="hy2")
                    nc.scalar.activation(hy2[:], y2[:], AF.Identity,
                                         bias=nmr[:], scale=rstd[:])
                    got = fp.tile([128, H], BF16, tag="got", bufs=2,
                                  name="got")
                    nc.vector.tensor_mul(got[:], hy2[:], g2b)
                    ot = fp.tile([128, H], F32, tag="ot", bufs=2, name="ot")
                    nc.vector.tensor_add(ot[:], got[:], be2b)
                    nc.sync.dma_start(out=out[qc, :, :], in_=ot[:])

    nc.compile()
    return nc


def _prep_host(inputs):
    """Fold scales/biases and build per-core input maps."""
    f = lambda k: np.asarray(inputs[k], dtype=np.float32)
    bf = ml_dtypes.bfloat16
    x = f("x")
    Wq_r, Wq_i = f("Wq_r"), f("Wq_i")
    bq_r, bq_i = f("bq_r"), f("bq_i")
    Wk_r, Wk_i = f("Wk_r"), f("Wk_i")
    bk_r, bk_i = f("bk_r"), f("bk_i")
    Wv, bv = f("Wv"), f("bv")
    pos_q_r, pos_q_i = f("pos_q_r"), f("pos_q_i")
    pos_k_r, pos_k_i = f("pos_k_r"), f("pos_k_i")
    Wo, bo = f("Wo"), f("bo")
    W1, b1 = f("W1"), f("b1")
    W2, b2 = f("W2"), f("b2")
    g1, beta1 = f("g1"), f("beta1")
    g2, beta2 = f("g2"), f("beta2")

    s2 = SCALE * SCALE
    Wq_cat = np.concatenate([Wq_r * s2, Wq_i * s2], axis=2)  # [N, H, 128]
    Wk_cat = np.concatenate([Wk_r, -Wk_i], axis=2)           # [N, H, 128]
    wq_dev = np.ascontiguousarray(
        Wq_cat.reshape(NH, HC, 128, 128).transpose(0, 2, 1, 3)
    ).reshape(NH, 128, HC * 128).astype(bf)
    wk_dev = np.ascontiguousarray(
        Wk_cat.reshape(NH, HC, 128, 128).transpose(0, 2, 1, 3)
    ).reshape(NH, 128, HC * 128).astype(bf)

    pq_eff = np.concatenate(
        [
            pos_q_r.transpose(0, 2, 1) * SCALE + (bq_r * s2)[:, :, None],
            pos_q_i.transpose(0, 2, 1) * SCALE + (bq_i * s2)[:, :, None],
        ],
        axis=1,
    )  # [N, 128, S]
    pk_eff = np.concatenate(
        [
            pos_k_r.transpose(0, 2, 1) + bk_r[:, :, None],
            -(pos_k_i.transpose(0, 2, 1) + bk_i[:, :, None]),
        ],
        axis=1,
    )  # [N, 128, S]

    wv_flat = Wv.transpose(1, 0, 2).reshape(H, NH * D)  # [H, 1024]
    wv_dev = np.ascontiguousarray(wv_flat.reshape(HC, 128, H)).astype(bf)
    wo_dev = np.ascontiguousarray(Wo.reshape(HC, 128, H)).astype(bf)
    bv_flat = bv.reshape(NH * D)
    bo_eff = bo + bv_flat @ Wo

    # fold LN1 affine into FFN entry: W1' = diag(g1)@W1, b1' = b1 + be1@W1
    W1p = W1 * g1[:, None]
    b1p = b1 + beta1 @ W1
    w1_dev = np.ascontiguousarray(
        W1p.reshape(HC, 128, FC, 128).transpose(2, 1, 0, 3)
    ).reshape(FC, 128, HC * 128).astype(bf)
    w2_dev = np.ascontiguousarray(W2.reshape(FC, 128, H)).astype(bf)
    b1_dev = np.ascontiguousarray(b1p.reshape(FC, 128).T)  # [128, FC]

    c1 = beta1 + b2
    cns_dev = np.ascontiguousarray(
        np.broadcast_to(
            np.stack([g1, c1, g2, beta2], axis=0)[None, :, :],
            (128, 4, H),
        )
    ).astype(bf)
    bce_dev = np.zeros((2, 128), np.float32)
    bce_dev[0, 0:64] = 1.0
    bce_dev[1, 64:128] = 1.0

    shared = {
        "wq": wq_dev, "wk": wk_dev, "wv": wv_dev, "wo": wo_dev,
        "posq": None, "posk": np.ascontiguousarray(pk_eff),
        "w1": w1_dev, "w2": w2_dev, "b1c": b1_dev, "cns": cns_dev,
        "bce": bce_dev,
    }

    in_maps = []
    for core in range(8):
        b, half = core // 2, core % 2
        qs = slice(half * SQ, (half + 1) * SQ)
        xTb = np.ascontiguousarray(
            x[b].T.reshape(HC, 128, S).transpose(1, 0, 2))  # [128, HC, S]
        xTqb = np.ascontiguousarray(xTb[:, :, qs])
        xq_plus = np.ascontiguousarray(
            (x[b, qs, :] + bo_eff[None, :]).reshape(QC, 128, H))
        m = dict(shared)
        m["posq"] = np.ascontiguousarray(pq_eff[:, :, qs])
        m["xT"] = xTb.astype(bf)
        m["xTq"] = xTqb.astype(bf)
        m["xqb"] = xq_plus
        in_maps.append(m)
    return in_maps


def kernel(**inputs) -> np.ndarray:
    if "nc" not in _CACHE:
        _CACHE["nc"] = build()
    nc = _CACHE["nc"]
    in_maps = _prep_host(inputs)
    res = run_bass_kernel_spmd(nc, in_maps, list(range(8)))
    outp = np.empty((B, S, H), np.float32)
    for core in range(8):
        b, half = core // 2, core % 2
        o = res.results[core]["out"].reshape(SQ, H)
        outp[b, half * SQ:(half + 1) * SQ, :] = o
    return outp


# revision 36
# speedup vs baseline: 1.1219x; 1.1219x over previous
"""Trainium2 Bass kernel for nn_ComplexEncoder (complex-QK transformer encoder layer).

Sharding: 8 cores = (batch b in 0..3) x (seq half h in 0..1). Each core
computes the full output rows for its (b, 512-row) slice. No collectives:
only the K/V projections are duplicated between the two cores of a batch.

v2 restructure (vs baseline): bf16 matmul operands everywhere (PE rate is
unchanged at 512-wide, but DMA/DVE/ACT all halve), softmax denominators
batched into one [16,512] reciprocal off the PE critical path, LayerNorm
via bn_stats + ACT-engine normalize with g1/beta1 folded into W1/b1 on the
host, adds/copies spread across GPSIMD/ACT so the PE queue never waits on
slow DVE work, exp output in bf16, W2 streamed once with 8 PSUM banks.

Math per core (b, half), all matmuls bf16 -> fp32 PSUM:
  qcat^T[128n+j, q] = scale^2-folded q-proj (+ pos folded on host)
  kcat^T[128n+j, k] = (Wk_r | -Wk_i) proj + (pos_k_r | -pos_k_i)
  scores^T[k, q]    = kcat_n^T.T-slices @ qcat_n  (single K=128 contraction)
  sT = exp(scores^T);  PV psum[0:65] via v_aug (col 64 = ones -> row sums)
  headsT = PV[0:64] * bcast(1/PV[64])  (batched recip + K=1 ones matmul)
  attn[q, h] = headsT.T @ Wo;  y1 = attn + (x + bo_eff);  yhat1 = LN1 core
  h1T = PE-transpose(yhat1); ffT = relu(W1'.T @ h1T + b1')   [g1 folded]
  y2 = ffT.T @ W2 + yhat1*g1 + (beta1+b2);  out = LN2(y2)*g2 + beta2
"""

import numpy as np
import ml_dtypes

import concourse.bass as bass
import concourse.bacc as bacc
import concourse.mybir as mybir
import concourse.tile as tile
from concourse.bass_utils import run_bass_kernel_spmd
from concourse.masks import make_identity

F32 = mybir.dt.float32
F32R = mybir.dt.float32r
BF16 = mybir.dt.bfloat16
AF = mybir.ActivationFunctionType
ALU = mybir.AluOpType
AX = mybir.AxisListType

B, S, H, NH, D, FF = 4, 1024, 1024, 16, 64, 4096
SQ = 512  # queries per core
EPS = 1e-5
SCALE = 1.0 / 8.0
HC = H // 128  # 8 chunks of the hidden/contraction dim
FC = FF // 128  # 32 chunks of the ff dim
QC = SQ // 128  # 4 query chunks
KC = S // 128  # 8 key chunks

_CACHE = {}


def build():
    nc = bacc.Bacc(
        "TRN2", target_bir_lowering=False, debug=False,
        enable_asserts=True, num_devices=8,
    )
    # --- DRAM parameters (host-prepped layouts) ---
    dp = nc.declare_dram_parameter
    xT = dp("xT", [128, HC, S], BF16, isOutput=False)       # x[b].T chunked
    xTq = dp("xTq", [128, HC, SQ], BF16, isOutput=False)    # q-half of xT
    wq = dp("wq", [NH, 128, HC * 128], BF16, isOutput=False)
    wk = dp("wk", [NH, 128, HC * 128], BF16, isOutput=False)
    wv = dp("wv", [HC, 128, H], BF16, isOutput=False)
    posq = dp("posq", [NH, 128, SQ], F32, isOutput=False)
    posk = dp("posk", [NH, 128, S], F32, isOutput=False)
    wo = dp("wo", [HC, 128, H], BF16, isOutput=False)
    xqb = dp("xqb", [QC, 128, H], F32, isOutput=False)      # x + bo_eff
    w1 = dp("w1", [FC, 128, HC * 128], BF16, isOutput=False)  # g1 folded
    w2 = dp("w2", [FC, 128, H], BF16, isOutput=False)
    b1c = dp("b1c", [128, FC], F32, isOutput=False)         # b1 + be1@W1
    cns = dp("cns", [128, 4, H], BF16, isOutput=False)      # g1,c1,g2,be2
    bce = dp("bce", [2, 128], F32R, isOutput=False)         # pair selector
    out = dp("out", [QC, 128, H], F32, isOutput=True)

    with tile.TileContext(nc) as tc:
        with (
            tc.tile_pool(name="const", bufs=1) as cp,
            tc.tile_pool(name="psum", bufs=1, space="PSUM") as pp,
            tc.tile_pool(name="persist", bufs=1) as lp,
        ):
            ident = cp.tile([128, 128], BF16)
            make_identity(nc, ident)
            eps_t = cp.tile([128, 1], F32)
            nc.vector.memset(eps_t, EPS)
            bceA = cp.tile([1, 128], F32R)
            nc.sync.dma_start(out=bceA, in_=bce[0:1, :])
            bceB = cp.tile([1, 128], F32R)
            nc.sync.dma_start(out=bceB, in_=bce[1:2, :])
            # b1/cns DMAs deferred below the first attention loads so the
            # head-0 weights win the DMA queue
            b1_sb = cp.tile([128, FC], F32)
            cns_sb = cp.tile([128, 4, H], BF16)
            g1b, c1b = cns_sb[:, 0, :], cns_sb[:, 1, :]
            g2b, be2b = cns_sb[:, 2, :], cns_sb[:, 3, :]

            hT_un = lp.tile([128, HC, SQ], BF16)   # unnormalized heads^T
            headsT = lp.tile([128, HC, SQ], BF16)  # normalized heads^T
            h1n = [lp.tile([128, H], BF16, tag="h1n", bufs=QC, name="h1n")
                   for _ in range(QC)]             # yhat1 (LN1 core out)
            h1gc = [lp.tile([128, H], F32, tag="h1gc", bufs=QC, name="h1gc")
                    for _ in range(QC)]            # yhat1*g1 + (be1+b2)
            h1T = lp.tile([128, HC, SQ], BF16)

            def ps():
                return pp.tile([128, 512], F32, tag="ps", bufs=8, name="pst")

            # ================= attention =================
            with tc.tile_pool(name="attn", bufs=1) as ap:
                xt_sb = ap.tile([128, HC, S], BF16)
                xtq_sb = ap.tile([128, HC, SQ], BF16)
                rr_r = [ap.tile([1, 512], F32R, tag="rr_r", bufs=NH,
                                name="rr_r") for _ in range(NH)]

                def dma_head(n):
                    wkt = ap.tile([128, HC * 128], BF16, tag="wkt", bufs=2,
                                  name="wkt")
                    nc.sync.dma_start(out=wkt, in_=wk[n, :, :])
                    wqt = ap.tile([128, HC * 128], BF16, tag="wqt", bufs=2,
                                  name="wqt")
                    nc.sync.dma_start(out=wqt, in_=wq[n, :, :])
                    pkt = ap.tile([128, S], F32, tag="pkt", bufs=2, name="pkt")
                    nc.sync.dma_start(out=pkt, in_=posk[n, :, :])
                    pqt = ap.tile([128, SQ], F32, tag="pqt", bufs=2,
                                  name="pqt")
                    nc.sync.dma_start(out=pqt, in_=posq[n, :, :])
                    return wkt, wqt, pkt, pqt

                def kq_thunks(wkt, wqt, pk0, pk1, pq):
                    th = []
                    for hc in range(HC):
                        st = wkt[:, hc * 128:(hc + 1) * 128]
                        th.append((pk0, st, xt_sb[:, hc, 0:512],
                                   hc == 0, hc == HC - 1))
                        th.append((pk1, st, xt_sb[:, hc, 512:1024],
                                   hc == 0, hc == HC - 1))
                    for hc in range(HC):
                        th.append((pq, wqt[:, hc * 128:(hc + 1) * 128],
                                   xtq_sb[:, hc, :], hc == 0, hc == HC - 1))
                    return th

                def run_thunk(t):
                    nc.tensor.matmul(t[0][:], t[1], t[2], start=t[3],
                                     stop=t[4])

                def kq_adds(pk0, pk1, pq, pkt, pqt):
                    # order: kcat half 1, qcat, kcat half 2 — the next head's
                    # first score matmul needs only the first two
                    kcat = ap.tile([128, S], BF16, tag="kcat", bufs=2,
                                   name="kcat")
                    nc.vector.tensor_add(kcat[:, 0:512], pk0[:], pkt[:, 0:512])
                    qcat = ap.tile([128, SQ], BF16, tag="qcat", bufs=2,
                                   name="qcat")
                    nc.vector.tensor_add(qcat[:], pq[:], pqt[:])
                    nc.vector.tensor_add(kcat[:, 512:1024], pk1[:],
                                         pkt[:, 512:1024])
                    return kcat, qcat

                def emit_pair(j):
                    # two K=1 outer products: head 2j's 1/denom row ->
                    # partitions 0:64, head 2j+1's -> 64:128 of one bank
                    pbc = ps()
                    nc.tensor.matmul(pbc[:], bceA[:], rr_r[2 * j][:],
                                     start=True, stop=False)
                    nc.tensor.matmul(pbc[:], bceB[:], rr_r[2 * j + 1][:],
                                     start=False, stop=True)
                    bc_sb = ap.tile([128, SQ], BF16, tag="bc", bufs=2,
                                    name="bc")
                    nc.scalar.copy(bc_sb[:], pbc[:])
                    nc.vector.tensor_mul(
                        headsT[:, j, :], hT_un[:, j, :], bc_sb[:])

                # prologue: head-0 weights first in the DMA queue, then x
                hw = dma_head(0)
                for hc in range(HC):
                    nc.sync.dma_start(out=xt_sb[:, hc, :], in_=xT[:, hc, :])
                for hc in range(HC):
                    nc.sync.dma_start(out=xtq_sb[:, hc, :], in_=xTq[:, hc, :])
                pk0, pk1, pq = ps(), ps(), ps()
                for t in kq_thunks(hw[0], hw[1], pk0, pk1, pq):
                    run_thunk(t)
                kcat, qcat = kq_adds(pk0, pk1, pq, hw[2], hw[3])
                # consts ride behind the startup-critical loads
                nc.sync.dma_start(out=b1_sb, in_=b1c[:, :])
                nc.sync.dma_start(out=cns_sb, in_=cns[:, :, :])

                # V projection (overlaps head-0 weight/pos DMAs)
                v_sb = []
                for kc in range(KC):
                    va = ap.tile([128, NH, 65], BF16, tag="vaug", bufs=KC,
                                 name="va")
                    v_sb.append(va)
                with tc.tile_pool(name="vw", bufs=1) as vwp:
                    wv_sb = []
                    for hc in range(HC):
                        wvt = vwp.tile([128, H], BF16, tag="wv", bufs=HC,
                                       name="wvt")
                        nc.sync.dma_start(out=wvt, in_=wv[hc, :, :])
                        wv_sb.append(wvt)
                    for kc in range(KC):
                        p0, p1 = ps(), ps()
                        for hc in range(HC):
                            st = xt_sb[:, hc, kc * 128:(kc + 1) * 128]
                            nc.tensor.matmul(p0[:], st, wv_sb[hc][:, 0:512],
                                             start=(hc == 0),
                                             stop=(hc == HC - 1))
                            nc.tensor.matmul(p1[:], st, wv_sb[hc][:, 512:1024],
                                             start=(hc == 0),
                                             stop=(hc == HC - 1))
                        nc.vector.tensor_copy(
                            v_sb[kc][:, 0:8, 0:64],
                            p0[:].rearrange("p (n d) -> p n d", d=64))
                        nc.scalar.copy(
                            v_sb[kc][:, 8:16, 0:64],
                            p1[:].rearrange("p (n d) -> p n d", d=64))
                        nc.gpsimd.memset(v_sb[kc][:, :, 64:65], 1.0)

                # per-head: scores/exp/PV for head n interleaved (in the PE
                # queue) with head n+1's K/Q projection matmuls
                pairq = []
                for n in range(NH):
                    last = (n == NH - 1)
                    th = []
                    if not last:
                        hw_n = dma_head(n + 1)
                        pk0n, pk1n, pqn = ps(), ps(), ps()
                        th = kq_thunks(hw_n[0], hw_n[1], pk0n, pk1n, pqn)
                    ppv = ps()
                    psc_prev = ps()
                    nc.tensor.matmul(psc_prev[:], kcat[:, 0:128], qcat[:],
                                     start=True, stop=True)
                    ti = 0
                    for kc in range(KC):
                        sT = ap.tile([128, SQ], BF16, tag="sT", bufs=3,
                                     name="sT")
                        nc.scalar.activation(sT[:], psc_prev[:], AF.Exp)
                        if kc < KC - 1:
                            psc_next = ps()
                            nc.tensor.matmul(
                                psc_next[:],
                                kcat[:, (kc + 1) * 128:(kc + 2) * 128],
                                qcat[:], start=True, stop=True)
                        nc.tensor.matmul(ppv[0:65, :], v_sb[kc][:, n, :],
                                         sT[:], start=(kc == 0),
                                         stop=(kc == KC - 1))
                        for _ in range(3):
                            if ti < len(th):
                                run_thunk(th[ti])
                                ti += 1
                        if kc < KC - 1:
                            psc_prev = psc_next
                    while ti < len(th):
                        run_thunk(th[ti])
                        ti += 1

                    off = (n % 2) * 64
                    nc.scalar.copy(hT_un[off:off + 64, n // 2, :],
                                   ppv[0:64, :])
                    if not last:
                        kcat, qcat = kq_adds(pk0n, pk1n, pqn, hw_n[2],
                                             hw_n[3])
                    rsc = ap.tile([1, 512], F32, tag="rsc", bufs=3,
                                  name="rsc")
                    nc.vector.reciprocal(rsc[:], ppv[64:65, :])
                    with nc.allow_low_precision(reason="denom fp32r"):
                        nc.gpsimd.tensor_copy(rr_r[n][:], rsc[:])
                    # normalize pairs trickle in two heads behind the recips
                    if n % 2 == 1:
                        pairq.append(n // 2)
                        if len(pairq) > 1:
                            emit_pair(pairq.pop(0))
                for j in pairq:
                    emit_pair(j)

            # ================= output proj + LN1 =================
            with tc.tile_pool(name="wop", bufs=1) as wop:
                wo_sb = []
                for j in range(HC):
                    wot = wop.tile([128, H], BF16, tag="wot", bufs=HC,
                                   name="wot")
                    nc.sync.dma_start(out=wot, in_=wo[j, :, :])
                    wo_sb.append(wot)

                def wo_block(qc):
                    po0, po1 = ps(), ps()
                    for j in range(HC):
                        st = headsT[:, j, qc * 128:(qc + 1) * 128]
                        nc.tensor.matmul(po0[:], st, wo_sb[j][:, 0:512],
                                         start=(j == 0), stop=(j == HC - 1))
                        nc.tensor.matmul(po1[:], st, wo_sb[j][:, 512:1024],
                                         start=(j == 0), stop=(j == HC - 1))
                    xqt = wop.tile([128, H], F32, tag="xqt", bufs=2,
                                   name="xqt")
                    nc.sync.dma_start(out=xqt, in_=xqb[qc, :, :])
                    y1 = wop.tile([128, H], F32, tag="y1", bufs=2, name="y1")
                    nc.vector.tensor_add(y1[:, 0:512], po0[:], xqt[:, 0:512])
                    nc.vector.tensor_add(y1[:, 512:1024], po1[:],
                                         xqt[:, 512:1024])
                    # LN1 core: yhat = (y1 - mu) * rstd  (g1/be1 folded out)
                    st6 = wop.tile([128, 2, 6], F32, tag="st6", bufs=2,
                                   name="st6")
                    nc.vector.bn_stats(st6[:, 0, :], y1[:, 0:512])
                    nc.vector.bn_stats(st6[:, 1, :], y1[:, 512:1024])
                    mv = wop.tile([128, 2], F32, tag="mv", bufs=2, name="mv")
                    nc.vector.bn_aggr(mv[:], st6[:])
                    std = wop.tile([128, 1], F32, tag="std", bufs=2,
                                   name="std")
                    nc.scalar.activation(std[:], mv[:, 1:2], AF.Sqrt,
                                         bias=eps_t[:])
                    rstd = wop.tile([128, 1], F32, tag="rstd", bufs=2,
                                    name="rstd")
                    nc.vector.reciprocal(rstd[:], std[:])
                    nmr = wop.tile([128, 1], F32, tag="nmr", bufs=2,
                                   name="nmr")
                    nc.vector.scalar_tensor_tensor(
                        nmr[:], mv[:, 0:1], -1.0, rstd[:],
                        op0=ALU.mult, op1=ALU.mult)
                    nc.scalar.activation(h1n[qc][:], y1[:], AF.Identity,
                                         bias=nmr[:], scale=rstd[:])
                    # residual term on idle gpsimd: h1gc = yhat*g1 + (be1+b2)
                    tres = wop.tile([128, H], BF16, tag="tres", bufs=2,
                                    name="tres")
                    nc.gpsimd.tensor_mul(tres[:], h1n[qc][:], g1b)
                    nc.gpsimd.tensor_add(h1gc[qc][:], tres[:], c1b)

                def tr_block(qc):
                    for j in range(HC):
                        pt = ps()
                        ptb = pt[:].bitcast(BF16)[:, 0:128]
                        nc.tensor.transpose(
                            ptb, h1n[qc][:, j * 128:(j + 1) * 128], ident[:])
                        nc.vector.tensor_copy(
                            h1T[:, j, qc * 128:(qc + 1) * 128], ptb)

                # stagger transposes two Wo blocks behind, hiding the LN chain
                wo_block(0)
                wo_block(1)
                wo_block(2)
                tr_block(0)
                wo_block(3)
                tr_block(1)
                tr_block(2)
                tr_block(3)

            # ================= FFN =================
            with tc.tile_pool(name="ffn", bufs=1) as fp:
                ffT = fp.tile([128, FC, SQ], BF16, name="ffT")
                w2sb = [fp.tile([128, H], BF16, tag="w2r", bufs=FC,
                                name="w2r") for _ in range(FC)]
                for f in range(FC):
                    w1t = fp.tile([128, HC * 128], BF16, tag="w1t", bufs=3,
                                  name="w1t")
                    nc.sync.dma_start(out=w1t, in_=w1[f, :, :])
                    # W2 row rides the DMA queue behind this step's W1 row,
                    # landing fully by the time FFN2 needs it
                    nc.sync.dma_start(out=w2sb[f], in_=w2[f, :, :])
                    pf = ps()
                    for hc in range(HC):
                        nc.tensor.matmul(pf[:], w1t[:, hc * 128:(hc + 1) * 128],
                                         h1T[:, hc, :],
                                         start=(hc == 0), stop=(hc == HC - 1))
                    nc.scalar.activation(ffT[:, f, :], pf[:], AF.Relu,
                                         bias=b1_sb[:, f:f + 1])

                # FFN2 per qc: each qc's LN2 tail hides under the next qc's
                # matmul stream
                for qc in range(QC):
                    py0, py1 = ps(), ps()
                    for f in range(FC):
                        st = ffT[:, f, qc * 128:(qc + 1) * 128]
                        nc.tensor.matmul(py0[:], st, w2sb[f][:, 0:512],
                                         start=(f == 0), stop=(f == FC - 1))
                        nc.tensor.matmul(py1[:], st, w2sb[f][:, 512:1024],
                                         start=(f == 0), stop=(f == FC - 1))
                    y2 = fp.tile([128, H], F32, tag="y2", bufs=2, name="y2")
                    nc.vector.tensor_add(y2[:, 0:512], py0[:],
                                         h1gc[qc][:, 0:512])
                    nc.vector.tensor_add(y2[:, 512:1024], py1[:],
                                         h1gc[qc][:, 512:1024])
                    st6 = fp.tile([128, 2, 6], F32, tag="st6b", bufs=2,
                                  name="st6b")
                    nc.vector.bn_stats(st6[:, 0, :], y2[:, 0:512])
                    nc.vector.bn_stats(st6[:, 1, :], y2[:, 512:1024])
                    mv = fp.tile([128, 2], F32, tag="mvb", bufs=2, name="mvb")
                    nc.vector.bn_aggr(mv[:], st6[:])
                    std = fp.tile([128, 1], F32, tag="stdb", bufs=2,
                                  name="stdb")
                    nc.scalar.activation(std[:], mv[:, 1:2], AF.Sqrt,
                                         bias=eps_t[:])
                    rstd = fp.tile([128, 1], F32, tag="rstdb", bufs=2,
                                   name="rstdb")
                    nc.vector.reciprocal(rstd[:], std[:])
                    nmr = fp.tile([128, 1], F32, tag="nmrb", bufs=2,
                                  name="nmrb")
                    nc.vector.scalar_tensor_tensor(
                        nmr[:], mv[:, 0:1], -1.0, rstd[:],
                        op0=ALU.mult, op1=ALU.mult)
                    hy2 = fp.tile([128, H], BF16, tag="hy2", bufs=2,
                                  name="hy2")
                    nc.scalar.activation(hy2[:], y2[:], AF.Identity,
                                         bias=nmr[:], scale=rstd[:])
                    got = fp.tile([128, H], BF16, tag="got", bufs=2,
                                  name="got")
                    nc.vector.tensor_mul(got[:], hy2[:], g2b)
                    ot = fp.tile([128, H], F32, tag="ot", bufs=2, name="ot")
                    nc.vector.tensor_add(ot[:], got[:], be2b)
                    nc.sync.dma_start(out=out[qc, :, :], in_=ot[:])

    nc.compile()
    return nc


def _prep_host(inputs):
    """Fold scales/biases and build per-core input maps."""
    f = lambda k: np.asarray(inputs[k], dtype=np.float32)
    bf = ml_dtypes.bfloat16
    x = f("x")
    Wq_r, Wq_i = f("Wq_r"), f("Wq_i")
    bq_r, bq_i = f("bq_r"), f("bq_i")
    Wk_r, Wk_i = f("Wk_r"), f("Wk_i")
    bk_r, bk_i = f("bk_r"), f("bk_i")
    Wv, bv = f("Wv"), f("bv")
    pos_q_r, pos_q_i = f("pos_q_r"), f("pos_q_i")
    pos_k_r, pos_k_i = f("pos_k_r"), f("pos_k_i")
    Wo, bo = f("Wo"), f("bo")
    W1, b1 = f("W1"), f("b1")
    W2, b2 = f("W2"), f("b2")
    g1, beta1 = f("g1"), f("beta1")
    g2, beta2 = f("g2"), f("beta2")

    s2 = SCALE * SCALE
    Wq_cat = np.concatenate([Wq_r * s2, Wq_i * s2], axis=2)  # [N, H, 128]
    Wk_cat = np.concatenate([Wk_r, -Wk_i], axis=2)           # [N, H, 128]
    wq_dev = np.ascontiguousarray(
        Wq_cat.reshape(NH, HC, 128, 128).transpose(0, 2, 1, 3)
    ).reshape(NH, 128, HC * 128).astype(bf)
    wk_dev = np.ascontiguousarray(
        Wk_cat.reshape(NH, HC, 128, 128).transpose(0, 2, 1, 3)
    ).reshape(NH, 128, HC * 128).astype(bf)

    pq_eff = np.concatenate(
        [
            pos_q_r.transpose(0, 2, 1) * SCALE + (bq_r * s2)[:, :, None],
            pos_q_i.transpose(0, 2, 1) * SCALE + (bq_i * s2)[:, :, None],
        ],
        axis=1,
    )  # [N, 128, S]
    pk_eff = np.concatenate(
        [
            pos_k_r.transpose(0, 2, 1) + bk_r[:, :, None],
            -(pos_k_i.transpose(0, 2, 1) + bk_i[:, :, None]),
        ],
        axis=1,
    )  # [N, 128, S]

    wv_flat = Wv.transpose(1, 0, 2).reshape(H, NH * D)  # [H, 1024]
    wv_dev = np.ascontiguousarray(wv_flat.reshape(HC, 128, H)).astype(bf)
    wo_dev = np.ascontiguousarray(Wo.reshape(HC, 128, H)).astype(bf)
    bv_flat = bv.reshape(NH * D)
    bo_eff = bo + bv_flat @ Wo

    # fold LN1 affine into FFN entry: W1' = diag(g1)@W1, b1' = b1 + be1@W1
    W1p = W1 * g1[:, None]
    b1p = b1 + beta1 @ W1
    w1_dev = np.ascontiguousarray(
        W1p.reshape(HC, 128, FC, 128).transpose(2, 1, 0, 3)
    ).reshape(FC, 128, HC * 128).astype(bf)
    w2_dev = np.ascontiguousarray(W2.reshape(FC, 128, H)).astype(bf)
    b1_dev = np.ascontiguousarray(b1p.reshape(FC, 128).T)  # [128, FC]

    c1 = beta1 + b2
    cns_dev = np.ascontiguousarray(
        np.broadcast_to(
            np.stack([g1, c1, g2, beta2], axis=0)[None, :, :],
            (128, 4, H),
        )
    ).astype(bf)
    bce_dev = np.zeros((2, 128), np.float32)
    bce_dev[0, 0:64] = 1.0
    bce_dev[1, 64:128] = 1.0

    shared = {
        "wq": wq_dev, "wk": wk_dev, "wv": wv_dev, "wo": wo_dev,
        "posq": None, "posk": np.ascontiguousarray(pk_eff),
        "w1": w1_dev, "w2": w2_dev, "b1c": b1_dev, "cns": cns_dev,
        "bce": bce_dev,
    }

    in_maps = []
    for core in range(8):
        b, half = core // 2, core % 2
        qs = slice(half * SQ, (half + 1) * SQ)
        xTb = np.ascontiguousarray(
            x[b].T.reshape(HC, 128, S).transpose(1, 0, 2))  # [128, HC, S]
        xTqb = np.ascontiguousarray(xTb[:, :, qs])
        xq_plus = np.ascontiguousarray(
            (x[b, qs, :] + bo_eff[None, :]).reshape(QC, 128, H))
        m = dict(shared)
        m["posq"] = np.ascontiguousarray(pq_eff[:, :, qs])
        m["xT"] = xTb.astype(bf)
        m["xTq"] = xTqb.astype(bf)
        m["xqb"] = xq_plus
        in_maps.append(m)
    return in_maps


def kernel(**inputs) -> np.ndarray:
    if "nc" not in _CACHE:
        _CACHE["nc"] = build()
    nc = _CACHE["nc"]
    in_maps = _prep_host(inputs)
    res = run_bass_kernel_spmd(nc, in_maps, list(range(8)))
    outp = np.empty((B, S, H), np.float32)
    for core in range(8):
        b, half = core // 2, core % 2
        o = res.results[core]["out"].reshape(SQ, H)
        outp[b, half * SQ:(half + 1) * SQ, :] = o
    return outp
